# revision 6
# baseline (speedup 1.0000x reference)
"""Quantized int8 matmul on 8 TRN2 NeuronCores.

Math: out = ((x - ZP_X) * SCALE_X) @ ((y - ZP_Y) * SCALE_Y)
Implemented as: out = [(x - ZP_X) @ (y - ZP_Y)] * (SCALE_X * SCALE_Y)
The zero-point-shifted int8 values (range ~[-150, 155]) are exactly
representable in bf16, so a bf16 matmul with fp32 PSUM accumulation is
numerically ~identical to the fp32 reference.

Sharding: x row-sharded (M) across 8 cores, y replicated, no collectives.
Per core: x_loc [512, 4096] i8, y [4096, 4096] i8 -> out_loc [512, 4096] f32.

Engine split per core:
  PE     - 1024 matmuls (128k x 128m x 512n), the only PE work
  DMA    - x/y loads, X-bar DMA-transpose of bf16 x into [k, m] layout, out store
  GpSimd - x int8->bf16 (+25) conversion (keeps DVE/ACT free at startup)
  DVE    - y int8->bf16 (-18) conversion (even k-tiles)
  ACT    - y conversion (odd k-tiles) + PSUM eviction with *SCALE_X*SCALE_Y
"""

import numpy as np

SCALE_X, ZP_X = 0.0215, -25
SCALE_Y, ZP_Y = 0.0176, 18
M, K, N = 4096, 4096, 4096
N_CORES = 8
P = 128
NBLK = 512  # matmul free dim = one PSUM bank of fp32
XCHUNK = 1024  # x conversion chunk (columns)


def build_nc(m_loc, k, n):
    from contextlib import ExitStack

    import concourse.mybir as mybir
    import concourse.tile as tile
    from concourse import bacc
    from concourse.bass import ts

    fp32 = mybir.dt.float32
    bf16 = mybir.dt.bfloat16
    int8 = mybir.dt.int8
    Copy = mybir.ActivationFunctionType.Copy

    MT = m_loc // P  # partition tiles of x rows
    KT = k // P  # contraction tiles
    NB = n // NBLK  # output column blocks
    xchunk = min(XCHUNK, k)  # x conversion chunk (columns)
    KC = k // xchunk  # x conversion chunks
    KPC = xchunk // P  # k-tiles per x chunk

    nc = bacc.Bacc(None, debug=False)
    x = nc.declare_dram_parameter("x", [m_loc, k], int8, isOutput=False)
    y = nc.declare_dram_parameter("y", [k, n], int8, isOutput=False)
    out = nc.declare_dram_parameter("out", [m_loc, n], fp32, isOutput=True)

    with ExitStack() as ctx:
        tc = ctx.enter_context(tile.TileContext(nc))
        xi_pool = ctx.enter_context(tc.tile_pool(name="xi", bufs=3))
        xb_pool = ctx.enter_context(tc.tile_pool(name="xb", bufs=3))
        xt_pool = ctx.enter_context(tc.tile_pool(name="xt", bufs=1))
        yi_pool = ctx.enter_context(tc.tile_pool(name="yi", bufs=8))
        yb_pool = ctx.enter_context(tc.tile_pool(name="yb", bufs=8))
        ob_pool = ctx.enter_context(tc.tile_pool(name="ob", bufs=4))
        ps_pool = ctx.enter_context(tc.tile_pool(name="ps", bufs=8, space="PSUM"))

        # Persistent transposed x: partition = k within tile, free = (kt, m)
        xT = xt_pool.tile([P, KT, m_loc], bf16)

        # Phase 1: load x, i8 -> bf16 with -ZP_X bias on GpSimd, X-bar
        # DMA-transpose (SBUF->SBUF, 2-byte dtype) into xT. No PE involved.
        for kc in range(KC):
            for mt in range(MT):
                xi = xi_pool.tile([P, xchunk], int8)
                nc.sync.dma_start(xi[:], x[ts(mt, P), ts(kc, xchunk)])
                xb = xb_pool.tile([P, xchunk], bf16)
                nc.gpsimd.tensor_scalar_add(xb[:], xi[:], float(-ZP_X))
                for kti in range(KPC):
                    kt = kc * KPC + kti
                    nc.sync.dma_start(
                        xT[:, kt, ts(mt, P)], xb[:, ts(kti, P)], transpose=True
                    )

        # Phase 2: stream y, accumulate in PSUM, scale on evict
        for nb in range(NB):
            psums = [
                ps_pool.tile([P, NBLK], fp32, tag="ps", name=f"acc_{nb}_{i}")
                for i in range(MT)
            ]
            for kt in range(KT):
                yi = yi_pool.tile([P, NBLK], int8)
                nc.sync.dma_start(yi[:], y[ts(kt, P), ts(nb, NBLK)])
                yb = yb_pool.tile([P, NBLK], bf16)
                if kt % 2 == 0:
                    nc.vector.tensor_scalar_add(yb[:], yi[:], float(-ZP_Y))
                else:
                    nc.scalar.activation(yb[:], yi[:], Copy, bias=float(-ZP_Y))
                for mt in range(MT):
                    nc.tensor.matmul(
                        psums[mt][:],
                        xT[:, kt, ts(mt, P)],
                        yb[:],
                        start=(kt == 0),
                        stop=(kt == KT - 1),
                    )
            for mt in range(MT):
                ob = ob_pool.tile([P, NBLK], fp32)
                nc.scalar.activation(
                    ob[:], psums[mt][:], Copy, scale=float(SCALE_X * SCALE_Y)
                )
                nc.sync.dma_start(out[ts(mt, P), ts(nb, NBLK)], ob[:])

    nc.compile()
    return nc


_NC_CACHE = None
LAST_RESULT = None  # BassKernelResults of the most recent run (for profiling)


def kernel(x, y):
    global _NC_CACHE, LAST_RESULT
    from concourse.bass_utils import run_bass_kernel_spmd

    x = np.asarray(x)
    y = np.asarray(y)
    assert x.shape == (M, K) and y.shape == (K, N), (x.shape, y.shape)
    x8 = x.astype(np.int8) if x.dtype != np.int8 else x
    y8 = y.astype(np.int8) if y.dtype != np.int8 else y

    if _NC_CACHE is None:
        _NC_CACHE = build_nc(M // N_CORES, K, N)
    nc = _NC_CACHE

    m_loc = M // N_CORES
    in_maps = [
        {"x": np.ascontiguousarray(x8[i * m_loc : (i + 1) * m_loc]), "y": y8}
        for i in range(N_CORES)
    ]
    res = run_bass_kernel_spmd(nc, in_maps, core_ids=list(range(N_CORES)))
    LAST_RESULT = res
    return np.concatenate(
        [np.asarray(res.results[i]["out"]) for i in range(N_CORES)], axis=0
    )


# revision 7
# speedup vs baseline: 1.7913x; 1.7913x over previous
"""Quantized int8 matmul on 8 TRN2 NeuronCores.

Math: out = ((x - ZP_X) * SCALE_X) @ ((y - ZP_Y) * SCALE_Y)
Implemented as: out = [(x - ZP_X) @ (y - ZP_Y)] * (SCALE_X * SCALE_Y)
The zero-point-shifted int8 values (range ~[-150, 155]) are exactly
representable in bf16, so a bf16 matmul with fp32 PSUM accumulation is
numerically ~identical to the fp32 reference.

Sharding: x row-sharded (M) across 8 cores, y replicated, no collectives.
Each core's x shard is laid out [K, m_loc] in DRAM (layout chosen at
shard time on host) so the TensorE stationary operand [k-part, m-free]
loads directly -- no on-device transpose.

Engine split per core:
  PE  - 1024 matmuls (128k x 128m x 512n), nothing else
  DMA - xT/y loads, out store
  DVE - int8->bf16 +bias conversions (even k-tiles)
  ACT - conversions (odd k-tiles) + PSUM eviction with *SCALE_X*SCALE_Y
"""

import numpy as np

SCALE_X, ZP_X = 0.0215, -25
SCALE_Y, ZP_Y = 0.0176, 18
M, K, N = 4096, 4096, 4096
N_CORES = 8
P = 128
NBLK = 512  # matmul free dim = one PSUM bank of fp32


def build_nc(m_loc, k, n):
    from contextlib import ExitStack

    import concourse.mybir as mybir
    import concourse.tile as tile
    from concourse import bacc
    from concourse.bass import ts

    fp32 = mybir.dt.float32
    bf16 = mybir.dt.bfloat16
    int8 = mybir.dt.int8
    Copy = mybir.ActivationFunctionType.Copy

    MT = m_loc // P  # m tiles (PE stationary free dim blocks)
    KT = k // P  # contraction tiles
    NB = n // NBLK  # output column blocks

    nc = bacc.Bacc(None, debug=False)
    xt = nc.declare_dram_parameter("xt", [k, m_loc], int8, isOutput=False)
    y = nc.declare_dram_parameter("y", [k, n], int8, isOutput=False)
    out = nc.declare_dram_parameter("out", [m_loc, n], fp32, isOutput=True)

    with ExitStack() as ctx:
        tc = ctx.enter_context(tile.TileContext(nc))
        xi_pool = ctx.enter_context(tc.tile_pool(name="xi", bufs=4))
        xt_pool = ctx.enter_context(tc.tile_pool(name="xtb", bufs=1))
        yi_pool = ctx.enter_context(tc.tile_pool(name="yi", bufs=8))
        yb_pool = ctx.enter_context(tc.tile_pool(name="yb", bufs=8))
        ob_pool = ctx.enter_context(tc.tile_pool(name="ob", bufs=4))
        ps_pool = ctx.enter_context(tc.tile_pool(name="ps", bufs=8, space="PSUM"))

        # Persistent bf16 x^T: partition = k within tile, free = (kt, m)
        xT = xt_pool.tile([P, KT, m_loc], bf16)

        def convert(dst, src, bias, on_dve):
            if on_dve:
                nc.vector.tensor_scalar_add(dst, src, bias)
            else:
                nc.scalar.activation(dst, src, Copy, bias=bias)

        # nb=0 interleaved with the x^T load+convert pipeline; both feed PE
        for nb in range(NB):
            psums = [
                ps_pool.tile([P, NBLK], fp32, tag="ps", name=f"acc_{nb}_{i}")
                for i in range(MT)
            ]
            for kt in range(KT):
                if nb == 0:
                    xi = xi_pool.tile([P, m_loc], int8)
                    nc.sync.dma_start(xi[:], xt[ts(kt, P), :])
                    convert(xT[:, kt, :], xi[:], float(-ZP_X), kt % 2 == 0)
                yi = yi_pool.tile([P, NBLK], int8)
                nc.sync.dma_start(yi[:], y[ts(kt, P), ts(nb, NBLK)])
                yb = yb_pool.tile([P, NBLK], bf16)
                convert(yb[:], yi[:], float(-ZP_Y), kt % 2 == 1)
                for mt in range(MT):
                    nc.tensor.matmul(
                        psums[mt][:],
                        xT[:, kt, ts(mt, P)],
                        yb[:],
                        start=(kt == 0),
                        stop=(kt == KT - 1),
                    )
            for mt in range(MT):
                ob = ob_pool.tile([P, NBLK], fp32)
                nc.scalar.activation(
                    ob[:], psums[mt][:], Copy, scale=float(SCALE_X * SCALE_Y)
                )
                nc.sync.dma_start(out[ts(mt, P), ts(nb, NBLK)], ob[:])

    nc.compile()
    return nc


_NC_CACHE = None
LAST_RESULT = None  # BassKernelResults of the most recent run (for profiling)


def kernel(x, y):
    global _NC_CACHE, LAST_RESULT
    from concourse.bass_utils import run_bass_kernel_spmd

    x = np.asarray(x)
    y = np.asarray(y)
    assert x.shape == (M, K) and y.shape == (K, N), (x.shape, y.shape)
    x8 = x.astype(np.int8) if x.dtype != np.int8 else x
    y8 = y.astype(np.int8) if y.dtype != np.int8 else y

    if _NC_CACHE is None:
        _NC_CACHE = build_nc(M // N_CORES, K, N)
    nc = _NC_CACHE

    m_loc = M // N_CORES
    in_maps = [
        {
            "xt": np.ascontiguousarray(x8[i * m_loc : (i + 1) * m_loc].T),
            "y": y8,
        }
        for i in range(N_CORES)
    ]
    res = run_bass_kernel_spmd(nc, in_maps, core_ids=list(range(N_CORES)))
    LAST_RESULT = res
    return np.concatenate(
        [np.asarray(res.results[i]["out"]) for i in range(N_CORES)], axis=0
    )


# revision 9
# speedup vs baseline: 2.2093x; 1.2333x over previous
"""Quantized int8 matmul on 8 TRN2 NeuronCores.

Math: out = ((x - ZP_X) * SCALE_X) @ ((y - ZP_Y) * SCALE_Y)
Implemented as: out = [(x - ZP_X) @ (y - ZP_Y)] * (SCALE_X * SCALE_Y)
The zero-point-shifted int8 values (range ~[-150, 155]) are exactly
representable in bf16, so a bf16 matmul with fp32 PSUM accumulation is
numerically ~identical to the fp32 reference.

Sharding: x row-sharded (M) across 8 cores, y replicated, no collectives.
Each core's x shard is laid out [K, m_loc] in DRAM (layout chosen at
shard time on host) so the TensorE stationary operand [k-part, m-free]
loads directly -- no on-device transpose.

Engine split per core:
  PE     - 1024 matmuls (128k x 128m x 512n), nothing else
  SP+ACT - HWDGE y loads (alternating) + out store
  GpSimd - SWDGE x loads (keeps the SP sequencer free)
  DVE    - x int8->bf16 +25, y conversion (even batches)
  ACT    - y conversion (odd batches) + PSUM eviction with *SCALE_X*SCALE_Y
"""

import numpy as np

SCALE_X, ZP_X = 0.0215, -25
SCALE_Y, ZP_Y = 0.0176, 18
M, K, N = 4096, 4096, 4096
N_CORES = 8
P = 128
NBLK = 512  # matmul free dim = one PSUM bank of fp32
KB = 2  # k-tiles per y DMA/convert batch
XB = 4  # k-tiles per x DMA/convert batch


def build_nc(m_loc, k, n):
    from contextlib import ExitStack

    import concourse.mybir as mybir
    import concourse.tile as tile
    from concourse import bacc
    from concourse.bass import ts

    fp32 = mybir.dt.float32
    bf16 = mybir.dt.bfloat16
    int8 = mybir.dt.int8
    Copy = mybir.ActivationFunctionType.Copy

    MT = m_loc // P  # m tiles (PE stationary free dim blocks)
    KT = k // P  # contraction tiles
    NB = n // NBLK  # output column blocks
    kb = min(KB, KT)  # y batch size in k-tiles
    xb = min(XB, KT)  # x batch size in k-tiles
    NKB = KT // kb
    x_every = max(1, xb // kb)  # one x batch per this many y batches

    nc = bacc.Bacc(None, debug=False)
    xt = nc.declare_dram_parameter("xt", [k, m_loc], int8, isOutput=False)
    y = nc.declare_dram_parameter("y", [k, n], int8, isOutput=False)
    out = nc.declare_dram_parameter("out", [m_loc, n], fp32, isOutput=True)

    # Batched DRAM views: group k into (batch, tile-in-batch, partition)
    xt_r = xt.rearrange("(g b p) m -> g p b m", b=xb, p=P)
    y_r = y.rearrange("(q b p) n -> q p b n", b=kb, p=P)

    with ExitStack() as ctx:
        tc = ctx.enter_context(tile.TileContext(nc))
        xi_pool = ctx.enter_context(tc.tile_pool(name="xi", bufs=3))
        xt_pool = ctx.enter_context(tc.tile_pool(name="xtb", bufs=1))
        yi_pool = ctx.enter_context(tc.tile_pool(name="yi", bufs=6))
        yb_pool = ctx.enter_context(tc.tile_pool(name="yb", bufs=6))
        ob_pool = ctx.enter_context(tc.tile_pool(name="ob", bufs=4))
        ps_pool = ctx.enter_context(tc.tile_pool(name="ps", bufs=8, space="PSUM"))

        # Persistent bf16 x^T: partition = k within tile, free = (kt, m)
        xT = xt_pool.tile([P, KT, m_loc], bf16)

        for nb in range(NB):
            psums = [
                ps_pool.tile([P, NBLK], fp32, tag="ps", name=f"acc_{nb}_{i}")
                for i in range(MT)
            ]
            for q in range(NKB):
                if nb == 0 and q % x_every == 0:
                    g = q // x_every
                    xi = xi_pool.tile([P, xb, m_loc], int8)
                    nc.gpsimd.dma_start(xi[:], xt_r[g])
                    nc.vector.tensor_scalar_add(
                        xT[:, ts(g, xb), :], xi[:], float(-ZP_X)
                    )
                yi = yi_pool.tile([P, kb, NBLK], int8)
                nc.sync.dma_start(yi[:], y_r[q, :, :, ts(nb, NBLK)])
                yb = yb_pool.tile([P, kb, NBLK], bf16)
                if q % 2 == 0:
                    nc.vector.tensor_scalar_add(yb[:], yi[:], float(-ZP_Y))
                else:
                    nc.scalar.activation(yb[:], yi[:], Copy, bias=float(-ZP_Y))
                for kti in range(kb):
                    kt = q * kb + kti
                    for mt in range(MT):
                        nc.tensor.matmul(
                            psums[mt][:],
                            xT[:, kt, ts(mt, P)],
                            yb[:, kti, :],
                            start=(kt == 0),
                            stop=(kt == KT - 1),
                        )
            for mt in range(MT):
                ob = ob_pool.tile([P, NBLK], fp32)
                nc.scalar.activation(
                    ob[:], psums[mt][:], Copy, scale=float(SCALE_X * SCALE_Y)
                )
                nc.sync.dma_start(out[ts(mt, P), ts(nb, NBLK)], ob[:])

    nc.compile()
    return nc


_NC_CACHE = None
LAST_RESULT = None  # BassKernelResults of the most recent run (for profiling)


def kernel(x, y):
    global _NC_CACHE, LAST_RESULT
    from concourse.bass_utils import run_bass_kernel_spmd

    x = np.asarray(x)
    y = np.asarray(y)
    assert x.shape == (M, K) and y.shape == (K, N), (x.shape, y.shape)
    x8 = x.astype(np.int8) if x.dtype != np.int8 else x
    y8 = y.astype(np.int8) if y.dtype != np.int8 else y

    if _NC_CACHE is None:
        _NC_CACHE = build_nc(M // N_CORES, K, N)
    nc = _NC_CACHE

    m_loc = M // N_CORES
    in_maps = [
        {
            "xt": np.ascontiguousarray(x8[i * m_loc : (i + 1) * m_loc].T),
            "y": y8,
        }
        for i in range(N_CORES)
    ]
    res = run_bass_kernel_spmd(nc, in_maps, core_ids=list(range(N_CORES)))
    LAST_RESULT = res
    return np.concatenate(
        [np.asarray(res.results[i]["out"]) for i in range(N_CORES)], axis=0
    )


# revision 10
# speedup vs baseline: 2.2309x; 1.0098x over previous
"""Quantized int8 matmul on 8 TRN2 NeuronCores.

Math: out = ((x - ZP_X) * SCALE_X) @ ((y - ZP_Y) * SCALE_Y)
Implemented as: out = [(x - ZP_X) @ (y - ZP_Y)] * (SCALE_X * SCALE_Y)
The zero-point-shifted int8 values (range ~[-150, 155]) are exactly
representable in bf16, so a bf16 matmul with fp32 PSUM accumulation is
numerically ~identical to the fp32 reference.

Sharding: x row-sharded (M) across 8 cores, y replicated, no collectives.
Each core's x shard is laid out [K, m_loc] in DRAM (layout chosen at
shard time on host) so the TensorE stationary operand [k-part, m-free]
loads directly -- no on-device transpose.

Engine split per core:
  PE     - warm-up dummies + 1024 matmuls (128k x 128m x 512n)
  SP     - HWDGE y loads, first x loads, out store
  GpSimd - SWDGE x loads (keeps the SP sequencer free), warmup memsets
  DVE    - x int8->bf16 +25 converts, y converts (even batches, nb>0),
           odd-mt PSUM evictions
  ACT    - y converts (all of nb=0; odd batches after) + even-mt evictions
"""

import numpy as np

SCALE_X, ZP_X = 0.0215, -25
SCALE_Y, ZP_Y = 0.0176, 18
M, K, N = 4096, 4096, 4096
N_CORES = 8
P = 128
NBLK = 512  # matmul free dim = one PSUM bank of fp32
KB = 2  # k-tiles per y DMA/convert batch
XB = 2  # k-tiles per x DMA/convert batch
N_WARM = 10  # PE warm-up dummy matmuls


def build_nc(m_loc, k, n):
    from contextlib import ExitStack

    import concourse.mybir as mybir
    import concourse.tile as tile
    from concourse import bacc
    from concourse.bass import ts

    fp32 = mybir.dt.float32
    bf16 = mybir.dt.bfloat16
    int8 = mybir.dt.int8
    Copy = mybir.ActivationFunctionType.Copy

    MT = m_loc // P  # m tiles (PE stationary free dim blocks)
    KT = k // P  # contraction tiles
    NB = n // NBLK  # output column blocks
    kb = min(KB, KT)  # y batch size in k-tiles
    xb = min(XB, KT)  # x batch size in k-tiles
    NKB = KT // kb
    NXB = KT // xb

    nc = bacc.Bacc(None, debug=False)
    xt = nc.declare_dram_parameter("xt", [k, m_loc], int8, isOutput=False)
    y = nc.declare_dram_parameter("y", [k, n], int8, isOutput=False)
    out = nc.declare_dram_parameter("out", [m_loc, n], fp32, isOutput=True)

    # Batched DRAM views: group k into (batch, tile-in-batch, partition)
    xt_r = xt.rearrange("(g b p) m -> g p b m", b=xb, p=P)
    y_r = y.rearrange("(q b p) n -> q p b n", b=kb, p=P)

    with ExitStack() as ctx:
        tc = ctx.enter_context(tile.TileContext(nc))
        wm_pool = ctx.enter_context(tc.tile_pool(name="wm", bufs=1))
        xi_pool = ctx.enter_context(tc.tile_pool(name="xi", bufs=4))
        xt_pool = ctx.enter_context(tc.tile_pool(name="xtb", bufs=1))
        yi_pool = ctx.enter_context(tc.tile_pool(name="yi", bufs=8))
        yb_pool = ctx.enter_context(tc.tile_pool(name="yb", bufs=8))
        ob_pool = ctx.enter_context(tc.tile_pool(name="ob", bufs=4))
        ps_pool = ctx.enter_context(tc.tile_pool(name="ps", bufs=8, space="PSUM"))

        # PE warm-up: a few dummy matmuls on zeroed tiles, issued during
        # the startup DMA window so the HAM clock-gate opens before the
        # real matmul stream begins.
        wm_w = wm_pool.tile([P, P], bf16)
        wm_s = wm_pool.tile([P, NBLK], bf16)
        nc.gpsimd.memset(wm_w[:], 0.0)
        nc.gpsimd.memset(wm_s[:], 0.0)
        ps_warm = ps_pool.tile([P, NBLK], fp32, tag="ps", name="warm")
        for _ in range(N_WARM):
            nc.tensor.matmul(ps_warm[:], wm_w[:], wm_s[:], start=True, stop=True)

        # Persistent bf16 x^T: partition = k within tile, free = (kt, m)
        xT = xt_pool.tile([P, KT, m_loc], bf16)

        def emit_x(g):
            if g >= NXB:
                return
            xi = xi_pool.tile([P, xb, m_loc], int8, name=f"xi_{g}")
            eng = nc.sync if g < 2 else nc.gpsimd
            eng.dma_start(xi[:], xt_r[g])
            nc.vector.tensor_scalar_add(xT[:, ts(g, xb), :], xi[:], float(-ZP_X))

        for nb in range(NB):
            psums = [
                ps_pool.tile([P, NBLK], fp32, tag="ps", name=f"acc_{nb}_{i}")
                for i in range(MT)
            ]
            for q in range(NKB):
                if nb == 0:
                    if q == 0:
                        emit_x(0)
                    emit_x(q + 1)
                yi = yi_pool.tile([P, kb, NBLK], int8)
                nc.sync.dma_start(yi[:], y_r[q, :, :, ts(nb, NBLK)])
                yb = yb_pool.tile([P, kb, NBLK], bf16)
                if nb > 0 and q % 2 == 0:
                    nc.vector.tensor_scalar_add(yb[:], yi[:], float(-ZP_Y))
                else:
                    nc.scalar.activation(yb[:], yi[:], Copy, bias=float(-ZP_Y))
                for kti in range(kb):
                    kt = q * kb + kti
                    for mt in range(MT):
                        nc.tensor.matmul(
                            psums[mt][:],
                            xT[:, kt, ts(mt, P)],
                            yb[:, kti, :],
                            start=(kt == 0),
                            stop=(kt == KT - 1),
                        )
            for mt in range(MT):
                ob = ob_pool.tile([P, NBLK], fp32)
                if mt % 2 == 0:
                    nc.scalar.activation(
                        ob[:], psums[mt][:], Copy, scale=float(SCALE_X * SCALE_Y)
                    )
                else:
                    nc.vector.tensor_scalar_mul(
                        ob[:], psums[mt][:], float(SCALE_X * SCALE_Y)
                    )
                nc.sync.dma_start(out[ts(mt, P), ts(nb, NBLK)], ob[:])

    nc.compile()
    return nc


_NC_CACHE = None
LAST_RESULT = None  # BassKernelResults of the most recent run (for profiling)


def kernel(x, y):
    global _NC_CACHE, LAST_RESULT
    from concourse.bass_utils import run_bass_kernel_spmd

    x = np.asarray(x)
    y = np.asarray(y)
    assert x.shape == (M, K) and y.shape == (K, N), (x.shape, y.shape)
    x8 = x.astype(np.int8) if x.dtype != np.int8 else x
    y8 = y.astype(np.int8) if y.dtype != np.int8 else y

    if _NC_CACHE is None:
        _NC_CACHE = build_nc(M // N_CORES, K, N)
    nc = _NC_CACHE

    m_loc = M // N_CORES
    in_maps = [
        {
            "xt": np.ascontiguousarray(x8[i * m_loc : (i + 1) * m_loc].T),
            "y": y8,
        }
        for i in range(N_CORES)
    ]
    res = run_bass_kernel_spmd(nc, in_maps, core_ids=list(range(N_CORES)))
    LAST_RESULT = res
    return np.concatenate(
        [np.asarray(res.results[i]["out"]) for i in range(N_CORES)], axis=0
    )
